# revision 2
# baseline (speedup 1.0000x reference)
"""Trainium2 Bass kernel for nn_MixedRepeatHeads.

Math (full shapes):
  proj[h,b,k] = einsum(x[b,d], proj_w[h,k,d]) + proj_b[h,k]
  w = mix_w[:, index]; bb = mix_b[:, index]
  decay = clip(decay_value, 0.9, 1.0) ** (1/8)
  coef[h] = w*decay (h<8) else decay
  hidden[b, i=h*256+k] = w[h]*proj[h,b,k] + coef[h]*cache[h,b,k] + bb[h]
  out = hidden @ out_w.T + out_b                       # [8192, 4096]

Weight folding on host (index-dependent scalars + weight-weight GEMM):
  PT[d,i] = w[h]*proj_w[h,k,d];  M = PT @ out_w.T
  C[b,i]  = coef[h]*cache[h,b,k]
  const_j = out_w @ (w*proj_b + bb)[i] + out_b
  out = XC @ W + const_j,  XC=[x|C] [8192,8192], W=[M;out_w.T] [8192,4096]

Device: data-parallel over batch (1024 rows/core).  The GEMM runs in fp8e4
DoubleRow mode at ~2x bf16 throughput: per k-tile one DR instruction whose
two slots hold
  slot0: x_hi * w_hi
  slot1: (x_lo + b*x_hi) * (c*w_hi + d*w_lo)      [combo correction]
with c = s, b = (s-1)/s, d = s^2/(s-1) so that
  slot0 + slot1 = s * (x @ w) + (s/(s-1)) * x_lo @ w_lo  (~u^2 error)
Final per j-tile: out = psum/(SX*SW*s) + const  (one Identity activation).
Measured end-to-end rel err vs fp32 reference: ~6.8e-3.
"""

import sys

if "/opt/trn_rl_repo" not in sys.path:
    sys.path.insert(0, "/opt/trn_rl_repo")

import numpy as np
import ml_dtypes

import bass_rust
import concourse.bass as bass
import concourse.tile as tile
from concourse import mybir
from concourse.bass_utils import run_bass_kernel_spmd
from concourse.vector_clock import ScopedClock

# ---------------------------------------------------------------- constants
N_HEADS = 16
HIDDEN = 256
DIM = 4096
BATCH = 8192
DECAY_CONSTANT = 8
N_CORES = 8
BC = BATCH // N_CORES  # 1024 batch rows per core
P = 128
DT = 32          # j-tiles along DIM
DT_K = 64        # k-tiles along the folded 8192 contraction
NFREE = 512      # PSUM free dim per matmul chain
HALF = BC // 2

F32 = mybir.dt.float32
F32R = mybir.dt.float32r
FP8 = mybir.dt.float8e4

SX = 16.0
SW = 64.0
S_COMBO = 1.15

# ------------------------------------------------- walrus wait legalization
# This walrus build supports only ONE sync-wait command per instruction.
MAXW = 1


class SafeTileContext(tile.TileContext):
    def _split_waits_in_ordered(self, ordered):
        nc = self.nc
        for _bb_name, insts in ordered.items():
            new_list = []
            changed = False
            for inst in insts:
                si = inst.sync_info
                if si is not None and len(si.on_wait) > MAXW:
                    waits = list(si.on_wait)
                    ups = list(si.on_update)
                    head, tail = waits[:-MAXW], waits[-MAXW:]
                    for w in head:
                        nop = mybir.InstNoOp(
                            name=nc.get_next_instruction_name(),
                            engine=inst.engine,
                            ins=[],
                            outs=[],
                            sync_info=bass_rust.SyncInfo(on_wait=[w], on_update=[]),
                            bass_nofuse=True,
                        )
                        nc.register_instruction(nop, overwrite=True)
                        new_list.append(nop)
                    inst.sync_info = bass_rust.SyncInfo(on_wait=tail, on_update=ups)
                    changed = True
                new_list.append(inst)
            if changed:
                insts[:] = new_list
        return ordered

    def _lower_ordered_insts(self, ordered):
        self._split_waits_in_ordered(ordered)
        return super()._lower_ordered_insts(ordered)

    def _drain_and_barrier(self, tick_clock, wait_clock):
        probe = self.nc.sync.nop(nofuse=True)
        wait_clock.add_sem_waits(
            probe.ins, ScopedClock({None: tick_clock.global_clock})
        )
        si = probe.ins.sync_info
        waits = list(si.on_wait) if si is not None else []
        upd = list(si.on_update) if si is not None else []
        probe.ins.sync_info = bass_rust.SyncInfo(on_wait=waits[:MAXW], on_update=upd)
        for i in range(MAXW, len(waits), MAXW):
            n = self.nc.sync.nop(nofuse=True)
            n.ins.sync_info = bass_rust.SyncInfo(
                on_wait=waits[i : i + MAXW], on_update=[]
            )

        self.nc.sync.drain()

        self.nc.all_engine_barrier()
        assert self.sems is not None
        popped = self.nc._tile_sem_poison_stack.pop()
        assert popped is self._sem_poison
        self.nc.clear_and_free_semaphores(list(self.sems.allocated().values()))
        self.nc.all_engine_barrier()


# ------------------------------------------------------------ kernel build
def build_kernel(probe_reuse_weights=False, loop_reps=None, deep_bufs=False):
    """Per-core program. DRAM params:
      xs  [P, DT_K*2*BC]    fp8 : xs[p, (kt*2+slot)*BC + b]; slot0=hi, slot1=combo
      wb  [DT, P, DT_K*2*P] fp8 : wb[jt, p, (kt*2+slot)*P + jj]
      cst [P, DT]           f32 : cst[p, jt] = const_j[jt*128+p]
      outT [DIM, BC]        f32 : output, j-major
    """
    nc = bass.Bass()
    xs = nc.declare_dram_parameter("xs", [P, DT_K * 2 * BC], FP8, isOutput=False)
    wb = nc.declare_dram_parameter("wb", [DT, P, DT_K * 2 * P], FP8, isOutput=False)
    cst = nc.declare_dram_parameter("cst", [P, DT], F32, isOutput=False)
    outT = nc.declare_dram_parameter("outT", [DIM, BC], F32, isOutput=True)

    n_chunks = BC // NFREE
    inv_scale = float(1.0 / (SX * SW * S_COMBO))

    with SafeTileContext(nc) as tc:
        with (
            tc.tile_pool(name="xpool", bufs=1) as xpool,
            tc.tile_pool(name="wpool", bufs=3) as wpool,
            tc.tile_pool(name="opool", bufs=4) as opool,
            tc.tile_pool(name="cpool", bufs=1) as cpool,
            tc.tile_pool(name="pspool", bufs=4, space="PSUM") as pspool,
        ):
            cst_t = cpool.tile([P, DT], F32)
            nc.sync.dma_start(cst_t[:], cst[:])

            import contextlib

            loop_cm = (
                tc.For_i(0, loop_reps, 1)
                if loop_reps is not None
                else contextlib.nullcontext()
            )
            with loop_cm:
                x_t = xpool.tile([P, DT_K, 2, BC], FP8, tag="x")
                XSPLIT = 8
                ks = DT_K // XSPLIT
                for sp in range(XSPLIT):
                    nc.sync.dma_start(
                        x_t[:, sp * ks : (sp + 1) * ks, :, :],
                        xs[:, sp * ks * 2 * BC : (sp + 1) * ks * 2 * BC].rearrange(
                            "p (k s b) -> p k s b", s=2, b=BC
                        ),
                    )

                for jt in range(DT):
                    w_t = wpool.tile([P, DT_K, 2, P], FP8, tag="w")
                    nc.sync.dma_start(
                        w_t[:],
                        wb[jt].rearrange("p (k s j) -> p k s j", s=2, j=P),
                    )
                    for ch in range(n_chunks):
                        bsl = slice(ch * NFREE, (ch + 1) * NFREE)
                        ps = pspool.tile([P, NFREE], F32, tag="ps")
                        for kt in range(DT_K):
                            nc.tensor.matmul(
                                ps[:],
                                w_t[:, kt, :, :],
                                x_t[:, kt, :, bsl],
                                start=(kt == 0),
                                stop=(kt == DT_K - 1),
                                perf_mode=mybir.MatmulPerfMode.DoubleRow,
                            )
                        o_t = opool.tile([P, NFREE], F32, tag="o")
                        nc.scalar.activation(
                            o_t[:],
                            ps[:],
                            mybir.ActivationFunctionType.Identity,
                            bias=cst_t[:, jt : jt + 1],
                            scale=inv_scale,
                        )
                        nc.sync.dma_start(
                            outT[jt * P : (jt + 1) * P, bsl], o_t[:]
                        )

    return nc


# ------------------------------------------------------------- host helpers
def _q8(a):
    return np.asarray(a, dtype=ml_dtypes.float8_e4m3)


def _host_prepare(inputs):
    x = np.asarray(inputs["x"], dtype=np.float32)
    proj_w = np.asarray(inputs["proj_w"], dtype=np.float32)
    proj_b = np.asarray(inputs["proj_b"], dtype=np.float32)
    mix_w = np.asarray(inputs["mix_w"], dtype=np.float32)
    mix_b = np.asarray(inputs["mix_b"], dtype=np.float32)
    decay_value = np.asarray(inputs["decay_value"], dtype=np.float32)
    cache = np.asarray(inputs["cache"], dtype=np.float32)
    out_w = np.asarray(inputs["out_w"], dtype=np.float32)
    out_b = np.asarray(inputs["out_b"], dtype=np.float32)
    idx = int(np.asarray(inputs["index"]))

    w = mix_w[:, idx]
    bb = mix_b[:, idx]
    decay = np.clip(decay_value, 0.9, 1.0) ** np.float32(1.0 / DECAY_CONSTANT)
    is_col = np.arange(N_HEADS) < (N_HEADS // 2)
    coef = np.where(is_col, w * decay, decay).astype(np.float32)

    # weight folding: M = (w*proj_w reshaped).T @ out_w.T
    pw = (proj_w * w[:, None, None]).reshape(DIM, DIM)  # [i, d]
    M = pw.T @ out_w.T  # [d, j]
    B2 = out_w.T
    W = np.concatenate([M, B2], axis=0).astype(np.float32)  # [8192, 4096]

    bias_i = (w[:, None] * proj_b + bb[:, None]).reshape(DIM)
    const_j = out_w @ bias_i + out_b

    C = np.ascontiguousarray(
        (coef[:, None, None] * cache).transpose(1, 0, 2).reshape(BATCH, DIM)
    )

    s = S_COMBO
    b_c = (s - 1.0) / s
    c_c = s
    d_c = s * s / (s - 1.0)

    Ws = W * SW
    W_hi8 = _q8(Ws)
    W_hi = W_hi8.astype(np.float32)
    W_cb8 = _q8(c_c * W_hi + d_c * (Ws - W_hi))

    def wtile(plane8):
        return plane8.reshape(DT_K, P, DT, P).transpose(2, 1, 0, 3)

    wb = np.empty((DT, P, DT_K, 2, P), dtype=ml_dtypes.float8_e4m3)
    wb[:, :, :, 0, :] = wtile(W_hi8)
    wb[:, :, :, 1, :] = wtile(W_cb8)
    wb = np.ascontiguousarray(wb.reshape(DT, P, DT_K * 2 * P))

    cstv = np.ascontiguousarray(const_j.reshape(DT, P).T)

    in_maps = []
    for cc in range(N_CORES):
        bsl = slice(cc * BC, (cc + 1) * BC)
        XCc = np.concatenate([x[bsl], C[bsl]], axis=1) * SX  # [BC, 8192]
        X_hi8 = _q8(XCc)
        X_hi = X_hi8.astype(np.float32)
        X_cb8 = _q8((XCc - X_hi) + b_c * X_hi)
        xsa = np.empty((P, DT_K, 2, BC), dtype=ml_dtypes.float8_e4m3)
        xsa[:, :, 0, :] = X_hi8.T.reshape(DT_K, P, BC).transpose(1, 0, 2)
        xsa[:, :, 1, :] = X_cb8.T.reshape(DT_K, P, BC).transpose(1, 0, 2)
        in_maps.append(
            {
                "xs": np.ascontiguousarray(xsa.reshape(P, DT_K * 2 * BC)),
                "wb": wb,
                "cst": cstv,
            }
        )
    return in_maps


def _assemble(results):
    out = np.empty((BATCH, DIM), dtype=np.float32)
    for c in range(N_CORES):
        out[c * BC : (c + 1) * BC] = results[c]["outT"].T
    return out


_NC_CACHE = None


def _get_nc():
    global _NC_CACHE
    if _NC_CACHE is None:
        _NC_CACHE = build_kernel()
    return _NC_CACHE


def kernel(**inputs) -> np.ndarray:
    in_maps = _host_prepare(inputs)
    nc = _get_nc()
    res = run_bass_kernel_spmd(nc, in_maps, list(range(N_CORES)))
    return _assemble(res.results)


if __name__ == "__main__":
    rng = np.random.default_rng(0)
    ins = {
        "x": rng.standard_normal((BATCH, DIM), dtype=np.float32),
        "proj_w": rng.standard_normal((N_HEADS, HIDDEN, DIM), dtype=np.float32) * 0.02,
        "proj_b": rng.standard_normal((N_HEADS, HIDDEN), dtype=np.float32) * 0.02,
        "mix_w": rng.standard_normal((N_HEADS, 4096), dtype=np.float32) * 0.02 + 1.0,
        "mix_b": rng.standard_normal((N_HEADS, 4096), dtype=np.float32) * 0.02,
        "decay_value": rng.uniform(0.85, 1.05, size=(N_HEADS,)).astype(np.float32),
        "cache": rng.standard_normal((N_HEADS, BATCH, HIDDEN), dtype=np.float32),
        "out_w": rng.standard_normal((DIM, DIM), dtype=np.float32) * 0.02,
        "out_b": rng.standard_normal((DIM,), dtype=np.float32) * 0.02,
        "index": 1000,
    }
    out = kernel(**ins)
    wv = ins["mix_w"][:, 1000]
    bbv = ins["mix_b"][:, 1000]
    dec = np.clip(ins["decay_value"], 0.9, 1.0) ** (1 / 8)
    coef = np.where(np.arange(16) < 8, wv * dec, dec)
    proj = np.einsum("bd,hkd->hbk", ins["x"], ins["proj_w"]) + ins["proj_b"][:, None, :]
    hidden = (
        wv[:, None, None] * proj
        + coef[:, None, None] * ins["cache"]
        + bbv[:, None, None]
    )
    hidden = hidden.transpose(1, 0, 2).reshape(BATCH, DIM)
    ref = hidden @ ins["out_w"].T + ins["out_b"]
    err = np.linalg.norm(out - ref) / np.linalg.norm(ref)
    print("out", out.shape, out.dtype, "rel err", err)
